# revision 1
# baseline (speedup 1.0000x reference)
"""HDModel retrieval kernel for 8x TRN2 NeuronCores.

reference:
    sims  = l2norm(hvs) @ l2norm(am).T        # [N, C] cosine sims
    preds = argmax(sims, axis=1)              # int32 [N]
    eta   = (sims[:,1]-sims[:,0])*0.25 + 0.5  # f32 [N]

Strategy (data-parallel over N, am replicated — no cross-core comms):
  - Host pre-transposes hvs -> hvsT [D, N/8] per shard and am -> amT [D, C]
    (layout staging only; all math happens on device).
  - sims are computed as raw = hvsT.T @ amT with f32r (tf32) matmuls,
    1 cyc/row on the PE at N>=256 vs fp32's 4.
  - am column norms (needed before argmax) via a bf16 ones-matmul over
    squared amT chunks; per-class scaling applied to sims rows on DVE.
  - hvs row norms (needed only for eta) via a bf16 gram matmul per n-tile;
    diagonal extracted with one DVE tensor_tensor_reduce against identity.
  - argmax via DVE max + max_index (top-8, index 0 = first-max like jnp).
  - preds/eta accumulate in [128, NT] tiles, one DMA out at the end;
    host reorders ([p, t] -> n = t*128+p).

This walrus build encodes ONE sync wait per TPB instruction; Tile attaches
several, so a post-pass splits multi-wait instructions into single-wait
same-engine NoOps (see _split_multiwait).
"""
import numpy as np
from contextlib import ExitStack

import concourse.bass as bass
import concourse.mybir as mybir
import concourse.tile as tile
from concourse.bass_utils import run_bass_kernel_spmd
from concourse.masks import make_identity

f32 = mybir.dt.float32
f32r = mybir.dt.float32r
bf16 = mybir.dt.bfloat16
u32 = mybir.dt.uint32

N_CORES = 8
N_FULL, D, C = 16384, 4096, 1024
NS = N_FULL // N_CORES          # 2048 rows per core
NT = NS // 128                  # 16 n-tiles
DCH = D // 128                  # 32 d-chunks
EPS = 1e-8


def _split_multiwait(nc):
    """Split multi-wait instructions into single-wait NoOps (walrus limit)."""
    ctr = [0]

    def mk_nop(engine, wait=None, update=None):
        ctr[0] += 1
        nop = mybir.InstNoOp(name=f"mwsplit_{ctr[0]}", ins=[], outs=[])
        nop.engine = engine
        nop.sync_info = mybir.SyncInfo(
            on_wait=[wait] if wait is not None else [],
            on_update=[update] if update is not None else [],
        )
        return nop

    for f in nc.m.functions:
        for bb in f.blocks:
            new = []
            changed = False
            for inst in bb.instructions:
                si = inst.sync_info
                if si is None:
                    new.append(inst)
                    continue
                waits = list(si.on_wait)
                updates = list(si.on_update)
                pre, post = [], []
                if len(waits) > 1:
                    pre = [mk_nop(inst.engine, wait=w) for w in waits[:-1]]
                    waits = waits[-1:]
                if len(updates) > 1 and type(inst).__name__ != "InstDMACopy":
                    post = [mk_nop(inst.engine, update=u) for u in updates[1:]]
                    updates = updates[:1]
                if pre or post:
                    inst.sync_info = mybir.SyncInfo(on_wait=waits, on_update=updates)
                    new.extend(pre)
                    new.append(inst)
                    new.extend(post)
                    changed = True
                else:
                    new.append(inst)
            if changed:
                bb.instructions = new


def build_nc():
    nc = bass.Bass()
    hvsT = nc.declare_dram_parameter("hvsT", [D, NS], f32r, isOutput=False)
    amT = nc.declare_dram_parameter("amT", [D, C], f32r, isOutput=False)
    ones_b = nc.declare_dram_parameter("ones_b", [128, 1], bf16, isOutput=False)
    ones_c = nc.declare_dram_parameter("ones_c", [1, 128], f32, isOutput=False)
    ident32 = nc.declare_dram_parameter("ident32", [128, 32], f32, isOutput=False)
    preds_o = nc.declare_dram_parameter("preds", [128, NT], u32, isOutput=True)
    eta_o = nc.declare_dram_parameter("eta", [128, NT], f32, isOutput=True)

    with tile.TileContext(nc) as tc, ExitStack() as ctx:
        const_p = ctx.enter_context(tc.tile_pool(name="const", bufs=1))
        am_p = ctx.enter_context(tc.tile_pool(name="am", bufs=1))
        sq_p = ctx.enter_context(tc.tile_pool(name="sq", bufs=3))
        hx_p = ctx.enter_context(tc.tile_pool(name="hx", bufs=2))
        hb_p = ctx.enter_context(tc.tile_pool(name="hb", bufs=2))
        ep_p = ctx.enter_context(tc.tile_pool(name="ep", bufs=2))
        acc_p = ctx.enter_context(tc.tile_pool(name="acc", bufs=1))
        ps_p = ctx.enter_context(tc.tile_pool(name="ps", bufs=2, space="PSUM"))
        psn_p = ctx.enter_context(tc.tile_pool(name="psn", bufs=1, space="PSUM"))

        # ---- constants ----
        ident = const_p.tile([128, 32], f32)
        nc.sync.dma_start(ident[:], ident32[:])
        ones_t = const_p.tile([128, 1], bf16)
        nc.sync.dma_start(ones_t[:], ones_b[:])
        ones_ct = const_p.tile([1, 128], f32)
        nc.sync.dma_start(ones_ct[:], ones_c[:])

        # ---- load amT (resident) ----
        am_tiles = []
        for dc in range(DCH):
            t = am_p.tile([128, C], f32r, tag=f"am{dc}")
            nc.sync.dma_start(t[:], amT[dc * 128:(dc + 1) * 128, :])
            am_tiles.append(t)

        # ---- am column norms: ones.T @ (amT**2), bf16 ----
        pn0 = psn_p.tile([1, 512], f32, tag="amn0")
        pn1 = psn_p.tile([1, 512], f32, tag="amn1")
        for dc in range(DCH):
            sq = sq_p.tile([128, C], bf16, tag="sq")
            nc.vector.tensor_mul(sq[:], am_tiles[dc][:].bitcast(f32),
                                 am_tiles[dc][:].bitcast(f32))
            nc.tensor.matmul(pn0[:], ones_t[:], sq[:, 0:512],
                             start=(dc == 0), stop=(dc == DCH - 1))
            nc.tensor.matmul(pn1[:], ones_t[:], sq[:, 512:C],
                             start=(dc == 0), stop=(dc == DCH - 1))

        # inv_c = 1 / max(sqrt(normsq), EPS), laid out [1, C] on partition 0
        amn = const_p.tile([1, C], f32)
        nc.scalar.sqrt(amn[:, 0:512], pn0[:])
        nc.scalar.sqrt(amn[:, 512:C], pn1[:])
        nc.vector.tensor_scalar_max(amn[:], amn[:], EPS)
        inv_c = const_p.tile([1, C], f32)
        nc.vector.reciprocal(inv_c[:], amn[:])

        # broadcast inv_c to all 128 partitions via exact fp32 ones-outer-product
        inv_cb = const_p.tile([128, C], f32)
        for h in range(2):
            bc = ps_p.tile([128, 512], f32, tag=("psA" if h == 0 else "psB"))
            nc.tensor.matmul(bc[:], ones_ct[:], inv_c[:, h * 512:(h + 1) * 512],
                             start=True, stop=True)
            nc.scalar.copy(inv_cb[:, h * 512:(h + 1) * 512], bc[:])

        # ---- accumulators ----
        preds_acc = acc_p.tile([128, NT], u32)
        eta_acc = acc_p.tile([128, NT], f32)

        # ---- main loop over n-tiles ----
        for t in range(NT):
            hx = hx_p.tile([128, D], f32r, tag="hx")
            src = hvsT[:, t * 128:(t + 1) * 128].rearrange(
                "(dc p) j -> p dc j", p=128)
            hxv = hx[:].rearrange("p (dc j) -> p dc j", j=128)
            half = DCH // 2
            nc.sync.dma_start(hxv[:, 0:half, :], src[:, 0:half, :])
            nc.sync.dma_start(hxv[:, half:DCH, :], src[:, half:DCH, :])

            hb = hb_p.tile([128, D], bf16, tag="hb")
            nc.scalar.copy(hb[:, 0:D // 2], hx[:, 0:D // 2].bitcast(f32))
            nc.scalar.copy(hb[:, D // 2:D], hx[:, D // 2:D].bitcast(f32))

            pA = ps_p.tile([128, 512], f32, tag="psA")
            pB = ps_p.tile([128, 512], f32, tag="psB")
            pG = ps_p.tile([128, 32], f32, tag="psG")
            for dc in range(DCH):
                lhs = hx[:, dc * 128:(dc + 1) * 128]
                nc.tensor.matmul(pA[:], lhs, am_tiles[dc][:, 0:512],
                                 start=(dc == 0), stop=(dc == DCH - 1))
                nc.tensor.matmul(pB[:], lhs, am_tiles[dc][:, 512:C],
                                 start=(dc == 0), stop=(dc == DCH - 1))
            # 4 col-packed 32-wide gram blocks run concurrently in the PE
            # array (tile_position col-tiling); only the diagonal is needed.
            for dc in range(DCH):
                for b in range(4):
                    sl = hb[:, dc * 128 + 32 * b:dc * 128 + 32 * (b + 1)]
                    nc.tensor.matmul(pG[32 * b:32 * (b + 1), :], sl, sl,
                                     start=(dc == 0), stop=(dc == DCH - 1),
                                     tile_position=(0, 32 * b))

            # epilogue
            sc = ep_p.tile([128, C], f32, tag="sc")
            nc.vector.tensor_mul(sc[:, 0:512], pA[:], inv_cb[:, 0:512])
            nc.vector.tensor_mul(sc[:, 512:C], pB[:], inv_cb[:, 512:C])

            dg = ep_p.tile([128, 32], f32, tag="dg")
            nsq = ep_p.tile([128, 1], f32, tag="nsq")
            nc.vector.tensor_mul(dg[:], pG[:], ident[:])
            nc.vector.reduce_sum(nsq[:], dg[:], axis=mybir.AxisListType.X)
            nrm = ep_p.tile([128, 1], f32, tag="nrm")
            nc.scalar.sqrt(nrm[:], nsq[:])
            nc.vector.tensor_scalar_max(nrm[:], nrm[:], EPS)
            inv_n = ep_p.tile([128, 1], f32, tag="invn")
            nc.vector.reciprocal(inv_n[:], nrm[:])

            mx = ep_p.tile([128, 8], f32, tag="mx")
            ix = ep_p.tile([128, 8], u32, tag="ix")
            nc.vector.max(out=mx[:], in_=sc[:])
            nc.vector.max_index(out=ix[:], in_max=mx[:], in_values=sc[:])
            nc.vector.tensor_copy(preds_acc[:, t:t + 1], ix[:, 0:1])

            d01 = ep_p.tile([128, 1], f32, tag="d01")
            nc.vector.tensor_sub(d01[:], sc[:, 1:2], sc[:, 0:1])
            nc.vector.tensor_mul(d01[:], d01[:], inv_n[:])
            nc.vector.tensor_scalar(
                out=eta_acc[:, t:t + 1], in0=d01[:], scalar1=0.25, scalar2=0.5,
                op0=mybir.AluOpType.mult, op1=mybir.AluOpType.add)

        nc.sync.dma_start(preds_o[:], preds_acc[:])
        nc.sync.dma_start(eta_o[:], eta_acc[:])

    _split_multiwait(nc)
    return nc


_CACHE = {}


def kernel(hvs: np.ndarray, am: np.ndarray):
    hvs = np.asarray(hvs, dtype=np.float32)
    am = np.asarray(am, dtype=np.float32)
    assert hvs.shape == (N_FULL, D) and am.shape == (C, D)

    if "nc" not in _CACHE:
        _CACHE["nc"] = build_nc()
    nc = _CACHE["nc"]

    amT = np.ascontiguousarray(am.T)                      # [D, C]
    import ml_dtypes
    ones_b = np.ones((128, 1), dtype=ml_dtypes.bfloat16)
    ones_c = np.ones((1, 128), dtype=np.float32)
    ident32 = np.zeros((128, 32), dtype=np.float32)
    for b in range(4):
        ident32[32 * b:32 * (b + 1), :] = np.eye(32, dtype=np.float32)

    in_maps = []
    for r in range(N_CORES):
        shard = hvs[r * NS:(r + 1) * NS]                  # [NS, D]
        hvsT = np.ascontiguousarray(shard.T)              # [D, NS]
        in_maps.append({"hvsT": hvsT, "amT": amT, "ones_b": ones_b,
                        "ones_c": ones_c, "ident32": ident32})

    res = run_bass_kernel_spmd(nc, in_maps, core_ids=list(range(N_CORES)))

    preds = np.empty(N_FULL, dtype=np.int32)
    eta = np.empty(N_FULL, dtype=np.float32)
    for r in range(N_CORES):
        p = res.results[r]["preds"]                       # [128, NT] u32
        e = res.results[r]["eta"]                         # [128, NT] f32
        preds[r * NS:(r + 1) * NS] = p.T.ravel().astype(np.int32)
        eta[r * NS:(r + 1) * NS] = e.T.ravel()
    return preds, eta



# revision 9
# speedup vs baseline: 1.1302x; 1.1302x over previous
"""HDModel retrieval kernel for 8x TRN2 NeuronCores.

reference:
    sims  = l2norm(hvs) @ l2norm(am).T        # [N, C] cosine sims
    preds = argmax(sims, axis=1)              # int32 [N]
    eta   = (sims[:,1]-sims[:,0])*0.25 + 0.5  # f32 [N]

Strategy (data-parallel over N, am replicated - no cross-core comms):
  - Host pre-transposes hvs -> hvsT [D, N/8] per shard and am -> amT [D, C]
    (layout staging only; all math happens on device).
  - sims computed as raw = hvsT.T @ amT with f32r (tf32) matmuls: the PE is
    the bottleneck engine at 16*32*1024 = 524288 col-streams (218.5us).
  - am column norms via ones-matmuls over ACT-squared am chunks; these are
    deliberately kept on the PE: they are am-gated filler that keeps the PE
    saturated during the serial am DMA window.
  - hvs row norms moved OFF the PE: ACT squares each hx half, DVE reduces
    over the dc axis (strided view), then a 4-cycle f32 matmul with a ones
    vector folds partitions; all 16 results live in one PSUM bank.
  - eta epilogue on ACT: inv4n = Rsqrt(16*nsq) (exact pow2 scale), then
    eta = d01*inv4n + 0.5 as a single scale(AP)+bias activation.
  - Emission order (= Tile scheduling priority) is computed by a small
    discrete-event estimator so each engine's in-order queue chases the
    serial 360GB/s DMA stream without head-of-line stalls: 3 n-tiles +
    am-norm chunks in flight while am streams, then tiles 3..15 run
    back-to-back PE-bound.

This walrus build encodes ONE sync wait per TPB instruction; Tile attaches
several, so a post-pass splits multi-wait instructions into single-wait
same-engine NoOps (see _split_multiwait).
"""
import numpy as np
from contextlib import ExitStack

import concourse.bass as bass
import concourse.mybir as mybir
import concourse.tile as tile
from concourse.bass_utils import run_bass_kernel_spmd

f32 = mybir.dt.float32
f32r = mybir.dt.float32r
bf16 = mybir.dt.bfloat16
u32 = mybir.dt.uint32

N_CORES = 8
N_FULL, D, C = 16384, 4096, 1024
NS = N_FULL // N_CORES          # 2048 rows per core
NT = NS // 128                  # 16 n-tiles
DCH = D // 128                  # 32 d-chunks
HD = DCH // 2                   # 16 d-chunks per hx half
T1 = 3                          # n-tiles in flight during the am window

# cost-model estimates (ns) used only to choose emission order
E_AM = 1456.0        # am chunk DMA [128,1024] f32
E_HX = 2913.0        # hx half DMA [128,2048] f32
E_MM = 427.0         # one 2x512-col f32r matmul pair at 2.4GHz
E_SQAM = 1100.0      # ACT square of an am chunk
E_SQHX = 1960.0      # ACT square of an hx half
E_RED = 2300.0       # DVE dc-reduce of one squared hx half
E_EPI = 3900.0       # DVE epilogue chain per tile


def _split_multiwait(nc):
    """Split multi-wait instructions into single-wait NoOps (walrus limit)."""
    ctr = [0]

    def mk_nop(engine, wait=None, update=None):
        ctr[0] += 1
        nop = mybir.InstNoOp(name=f"mwsplit_{ctr[0]}", ins=[], outs=[])
        nop.engine = engine
        nop.sync_info = mybir.SyncInfo(
            on_wait=[wait] if wait is not None else [],
            on_update=[update] if update is not None else [],
        )
        return nop

    for f in nc.m.functions:
        for bb in f.blocks:
            new = []
            changed = False
            for inst in bb.instructions:
                si = inst.sync_info
                if si is None:
                    new.append(inst)
                    continue
                waits = list(si.on_wait)
                updates = list(si.on_update)
                pre, post = [], []
                if len(waits) > 1:
                    pre = [mk_nop(inst.engine, wait=w) for w in waits[:-1]]
                    waits = waits[-1:]
                if len(updates) > 1 and type(inst).__name__ != "InstDMACopy":
                    post = [mk_nop(inst.engine, update=u) for u in updates[1:]]
                    updates = updates[:1]
                if pre or post:
                    inst.sync_info = mybir.SyncInfo(on_wait=waits, on_update=updates)
                    new.extend(pre)
                    new.append(inst)
                    new.extend(post)
                    changed = True
                else:
                    new.append(inst)
            if changed:
                bb.instructions = new


def _phase1_dma_order():
    """Serial-DMA item order for the am window: hx halves for tiles 0..2
    threaded into the am chunk stream so the PE always has fresh work."""
    order = ["h0A", "a0", "a1", "h0B", "a2", "a3", "h1A", "a4", "h1B",
             "a5", "a6", "h2A", "a7", "h2B"]
    order += [f"a{dc}" for dc in range(8, DCH)]
    return order


def build_nc():
    nc = bass.Bass()
    hvsT = nc.declare_dram_parameter("hvsT", [D, NS], f32r, isOutput=False)
    amT = nc.declare_dram_parameter("amT", [D, C], f32r, isOutput=False)
    ones_b = nc.declare_dram_parameter("ones_b", [128, 1], bf16, isOutput=False)
    ones_f = nc.declare_dram_parameter("ones_f", [128, 1], f32, isOutput=False)
    ones_c = nc.declare_dram_parameter("ones_c", [1, 128], f32, isOutput=False)
    preds_o = nc.declare_dram_parameter("preds", [128, NT], u32, isOutput=True)
    eta_o = nc.declare_dram_parameter("eta", [128, NT], f32, isOutput=True)

    with tile.TileContext(nc) as tc, ExitStack() as ctx:
        const_p = ctx.enter_context(tc.tile_pool(name="const", bufs=1))
        am_p = ctx.enter_context(tc.tile_pool(name="am", bufs=1))
        hx_p = ctx.enter_context(tc.tile_pool(name="hx", bufs=6))
        sqam_p = ctx.enter_context(tc.tile_pool(name="sqam", bufs=2))
        sqh_p = ctx.enter_context(tc.tile_pool(name="sqh", bufs=1))
        sqr_p = ctx.enter_context(tc.tile_pool(name="sqr", bufs=4))
        sc_p = ctx.enter_context(tc.tile_pool(name="sc", bufs=1))
        ep_p = ctx.enter_context(tc.tile_pool(name="ep", bufs=2))
        acc_p = ctx.enter_context(tc.tile_pool(name="acc", bufs=1))
        pair_p = ctx.enter_context(tc.tile_pool(name="pair", bufs=T1, space="PSUM"))
        psn_p = ctx.enter_context(tc.tile_pool(name="psn", bufs=1, space="PSUM"))
        psr_p = ctx.enter_context(tc.tile_pool(name="psr", bufs=1, space="PSUM"))

        # ---------- event list: (est_start_ns, seq, emit_fn) ----------
        ev = []

        def emit(t_est, fn):
            ev.append((t_est, len(ev), fn))

        eng = {"dma": 0.0, "pe": 0.0, "act": 0.0, "dve": 0.0}

        def sched(engine, ready, dur, fn):
            t0 = max(eng[engine], ready)
            eng[engine] = t0 + dur
            emit(t0, fn)
            return t0 + dur

        # ---------- constants ----------
        ones_t = const_p.tile([128, 1], bf16)
        ones_ft = const_p.tile([128, 1], f32)
        ones_ct = const_p.tile([1, 128], f32)
        sched("dma", 0.0, 60.0, lambda: nc.sync.dma_start(ones_t[:], ones_b[:]))
        sched("dma", 0.0, 10.0, lambda: nc.sync.dma_start(ones_ft[:], ones_f[:]))
        sched("dma", 0.0, 10.0, lambda: nc.sync.dma_start(ones_ct[:], ones_c[:]))

        # ---------- tiles ----------
        am_tiles = [am_p.tile([128, C], f32r, tag=f"am{dc}", name=f"am{dc}") for dc in range(DCH)]
        hx_half = {}                      # (t, h) -> tile
        inv_cb = const_p.tile([128, C], f32)
        inv_c = const_p.tile([1, C], f32)
        preds_acc = acc_p.tile([128, NT], u32)
        eta_acc = acc_p.tile([128, NT], f32)
        bR = psr_p.tile([128, NT], f32)   # all tiles' row-norm sums

        am_arr = {}                       # dc -> est arrival
        hx_arr = {}                       # (t, h) -> est arrival
        sqam_done = {}                    # dc -> est
        red_done = {}                     # t -> est both halves reduced
        tiny_done = {}                    # t -> est
        mm_done = {}                      # t -> est of stop matmul
        scmul_done = {}                   # t -> est pair freed
        d01_done = {}                     # t -> est

        def dma_am(dc):
            def fn():
                nc.sync.dma_start(am_tiles[dc][:], amT[dc * 128:(dc + 1) * 128, :])
            am_arr[dc] = sched("dma", 0.0, E_AM, fn)

        def dma_hx(t, h):
            ht = hx_p.tile([128, HD * 128], f32r, tag="hx", name=f"hx{t}_{h}")
            hx_half[(t, h)] = ht

            def fn():
                src = hvsT[h * 2048:(h + 1) * 2048,
                           t * 128:(t + 1) * 128].rearrange(
                    "(dc p) j -> p dc j", p=128)
                nc.sync.dma_start(
                    ht[:].rearrange("p (dc j) -> p dc j", j=128), src)
            hx_arr[(t, h)] = sched("dma", 0.0, E_HX, fn)

        # ACT square of am chunk + DVE-free: feeds the pn filler matmuls
        sq_tiles = {}

        def act_sqam(dc):
            sq = sqam_p.tile([128, C], bf16, tag="sqam", name=f"sqam{dc}")
            sq_tiles[dc] = sq

            def fn():
                nc.scalar.square(sq[:], am_tiles[dc][:].bitcast(f32))
            sqam_done[dc] = sched("act", am_arr[dc], E_SQAM, fn)

        # hvs row-norm pipeline: ACT square half -> DVE reduce dc -> tiny mm
        def rownorm_half(t, h):
            ht = hx_half[(t, h)]
            sq = sqh_p.tile([128, HD * 128], bf16, tag="sqh", name=f"sqh{t}_{h}")
            sqr = sqr_p.tile([128, 128], f32, tag="sqr", name=f"sqr{t}_{h}")

            def fn_sq():
                nc.scalar.square(sq[:], ht[:].bitcast(f32))
            t_sq = sched("act", hx_arr[(t, h)], E_SQHX, fn_sq)

            def fn_red():
                nc.vector.reduce_sum(
                    sqr[:], sq[:].rearrange("p (dc j) -> p j dc", j=128),
                    axis=mybir.AxisListType.X)
            t_red = sched("dve", t_sq, E_RED, fn_red)
            red_done[t] = max(red_done.get(t, 0.0), t_red)
            return sqr

        def rownorm_mms(t, sqrA, sqrB):
            def fnA():
                nc.tensor.matmul(bR[:, t:t + 1], sqrA[:], ones_ft[:],
                                 start=True, stop=False)

            def fnB():
                nc.tensor.matmul(bR[:, t:t + 1], sqrB[:], ones_ft[:],
                                 start=False, stop=True)
            t0 = sched("pe", max(red_done[t], mm_done.get(t, 0.0)), 10.0, fnA)
            tiny_done[t] = sched("pe", t0, 10.0, fnB)

        # main matmul pair for (t, dc)
        pairs = {}

        def get_pair(t):
            if t not in pairs:
                pairs[t] = pair_p.tile([128, C], f32, tag="pair", name=f"pair{t}")
            return pairs[t]

        def mm(t, dc):
            pt = get_pair(t)
            ht = hx_half[(t, dc // HD)]
            lhs = ht[:, (dc % HD) * 128:(dc % HD + 1) * 128]

            def fnA():
                nc.tensor.matmul(pt[:, 0:512], lhs, am_tiles[dc][:, 0:512],
                                 start=(dc == 0), stop=(dc == DCH - 1))

            def fnB():
                nc.tensor.matmul(pt[:, 512:C], lhs, am_tiles[dc][:, 512:C],
                                 start=(dc == 0), stop=(dc == DCH - 1))
            ready = max(am_arr[dc], hx_arr[(t, dc // HD)],
                        scmul_done.get(t - T1, 0.0))
            t0 = sched("pe", ready, E_MM / 2, fnA)
            mm_done[t] = sched("pe", t0, E_MM / 2, fnB)

        # am-norm filler matmuls (pn accumulation over dc)
        pn = psn_p.tile([128, 512], f32)

        def pn_mm(dc):
            sq = sq_tiles[dc]

            def fnA():
                nc.tensor.matmul(pn[0:1, :], ones_t[:], sq[:, 0:512],
                                 start=(dc == 0), stop=(dc == DCH - 1))

            def fnB():
                nc.tensor.matmul(pn[32:33, :], ones_t[:], sq[:, 512:C],
                                 start=(dc == 0), stop=(dc == DCH - 1))
            t0 = sched("pe", sqam_done[dc], E_MM / 2, fnA)
            return sched("pe", t0, E_MM / 2, fnB)

        # ---------- phase 1: am window with tiles 0..T1-1 in flight ----------
        sqrs = {t: [None, None] for t in range(T1)}
        for item in _phase1_dma_order():
            if item[0] == "a":
                dc = int(item[1:])
                dma_am(dc)
                act_sqam(dc)
            else:
                t, h = int(item[1:-1]), (0 if item[-1] == "A" else 1)
                dma_hx(t, h)
                sqrs[t][h] = rownorm_half(t, h)

        # chase-order the phase-1 PE stream: (t, dc) mains + pn fillers
        cand = [("mm", t, dc) for t in range(T1) for dc in range(DCH)]
        cand += [("pn", 0, dc) for dc in range(DCH)]

        def ready_of(kind, t, dc):
            if kind == "pn":
                return sqam_done[dc]
            return max(am_arr[dc], hx_arr[(t, dc // HD)])

        cand.sort(key=lambda c: (ready_of(*c), c[2], c[1]))
        pn_end = 0.0
        for kind, t, dc in cand:
            if kind == "pn":
                pn_end = pn_mm(dc)
            else:
                mm(t, dc)

        # ---------- boundary: inv_c pipeline + phase-1 tiny row-norm mms ----
        for t in range(T1):
            rownorm_mms(t, sqrs[t][0], sqrs[t][1])

        amn = inv_cb[0:1, :]  # scratch overlay; overwritten later by bcast

        def fn_sqrt_pn0():
            nc.scalar.sqrt(amn[:, 0:512], pn[0:1, :])

        def fn_sqrt_pn1():
            nc.scalar.sqrt(amn[:, 512:C], pn[32:33, :])

        def fn_recip_pn():
            nc.vector.reciprocal(inv_c[:], amn[:])
        t_invc = sched("act", pn_end + 200.0, 500.0, fn_sqrt_pn0)
        t_invc = sched("act", t_invc, 500.0, fn_sqrt_pn1)
        t_invc = sched("dve", t_invc + 100.0, 1200.0, fn_recip_pn)

        t_bc = t_invc
        for h in range(2):
            def fn_bc(h=h):
                nc.tensor.matmul(pn[:], ones_ct[:],
                                 inv_c[:, h * 512:(h + 1) * 512],
                                 start=True, stop=True)

            def fn_cp(h=h):
                nc.scalar.copy(inv_cb[:, h * 512:(h + 1) * 512], pn[:])
            t_mm0 = sched("pe", t_bc, 860.0, fn_bc)
            t_bc = sched("act", t_mm0 + 200.0, 600.0, fn_cp)
        inv_cb_done = t_bc

        # ---------- epilogue (shared) ----------
        def epilogue(t):
            pt = pairs.pop(t)
            sc = sc_p.tile([128, C], f32, tag="sc", name=f"sc{t}")
            mx = ep_p.tile([128, 8], f32, tag="mx", name=f"mx{t}")
            ix = ep_p.tile([128, 8], u32, tag="ix", name=f"ix{t}")
            d01 = ep_p.tile([128, 1], f32, tag="d01", name=f"d01{t}")
            inv4n = ep_p.tile([128, 1], f32, tag="inv4n", name=f"inv4n{t}")
            ready = max(mm_done[t] + 200.0, inv_cb_done)

            def fn_sc0():
                nc.vector.tensor_mul(sc[:, 0:512], pt[:, 0:512], inv_cb[:, 0:512])

            def fn_sc1():
                nc.vector.tensor_mul(sc[:, 512:C], pt[:, 512:C], inv_cb[:, 512:C])
            t0 = sched("dve", ready, 800.0, fn_sc0)
            scmul_done[t] = sched("dve", t0, 800.0, fn_sc1)

            def fn_max():
                nc.vector.max(out=mx[:], in_=sc[:])

            def fn_mix():
                nc.vector.max_index(out=ix[:], in_max=mx[:], in_values=sc[:])

            def fn_pc():
                nc.vector.tensor_copy(preds_acc[:, t:t + 1], ix[:, 0:1])

            def fn_d01():
                nc.vector.tensor_sub(d01[:], sc[:, 1:2], sc[:, 0:1])
            t0 = sched("dve", scmul_done[t], 1200.0, fn_max)
            t0 = sched("dve", t0, 1200.0, fn_mix)
            t0 = sched("dve", t0, 100.0, fn_pc)
            d01_done[t] = sched("dve", t0, 100.0, fn_d01)

            nrm4 = ep_p.tile([128, 1], f32, tag="nrm4", name=f"nrm4_{t}")

            def fn_sq4():
                nc.scalar.activation(nrm4[:], bR[:, t:t + 1],
                                     mybir.ActivationFunctionType.Sqrt,
                                     scale=16.0)

            def fn_rec4():
                nc.vector.reciprocal(inv4n[:], nrm4[:])

            def fn_eta():
                nc.scalar.activation(eta_acc[:, t:t + 1], d01[:],
                                     mybir.ActivationFunctionType.Copy,
                                     bias=0.5, scale=inv4n[:])
            t0 = sched("act", tiny_done[t] + 200.0, 300.0, fn_sq4)
            t0 = sched("dve", max(t0, d01_done[t]), 150.0, fn_rec4)
            sched("act", t0 + 150.0, 300.0, fn_eta)

        for t in range(T1):
            epilogue(t)

        # ---------- phase 2: tiles T1..NT-1, PE-bound ----------
        for t in range(T1, NT):
            dma_hx(t, 0)
            dma_hx(t, 1)
            sqrA = rownorm_half(t, 0)
            sqrB = rownorm_half(t, 1)
            for dc in range(DCH):
                mm(t, dc)
            rownorm_mms(t, sqrA, sqrB)
            epilogue(t)

        # ---------- outputs ----------
        def fn_po():
            nc.sync.dma_start(preds_o[:], preds_acc[:])

        def fn_eo():
            nc.sync.dma_start(eta_o[:], eta_acc[:])
        t_end = max(eng["dve"], eng["act"])
        sched("dma", t_end, 100.0, fn_po)
        sched("dma", t_end, 100.0, fn_eo)

        # ---------- emit in estimated-start order ----------
        ev.sort(key=lambda e: (e[0], e[1]))
        for _, _, fn in ev:
            fn()

    _split_multiwait(nc)
    return nc


_CACHE = {}


def kernel(hvs: np.ndarray, am: np.ndarray):
    hvs = np.asarray(hvs, dtype=np.float32)
    am = np.asarray(am, dtype=np.float32)
    assert hvs.shape == (N_FULL, D) and am.shape == (C, D)

    if "nc" not in _CACHE:
        _CACHE["nc"] = build_nc()
    nc = _CACHE["nc"]

    amT = np.ascontiguousarray(am.T)                      # [D, C]
    import ml_dtypes
    ones_b = np.ones((128, 1), dtype=ml_dtypes.bfloat16)
    ones_f = np.ones((128, 1), dtype=np.float32)
    ones_c = np.ones((1, 128), dtype=np.float32)

    in_maps = []
    for r in range(N_CORES):
        shard = hvs[r * NS:(r + 1) * NS]                  # [NS, D]
        hvsT = np.ascontiguousarray(shard.T)              # [D, NS]
        in_maps.append({"hvsT": hvsT, "amT": amT, "ones_b": ones_b,
                        "ones_f": ones_f, "ones_c": ones_c})

    res = run_bass_kernel_spmd(nc, in_maps, core_ids=list(range(N_CORES)))

    preds = np.empty(N_FULL, dtype=np.int32)
    eta = np.empty(N_FULL, dtype=np.float32)
    for r in range(N_CORES):
        p = res.results[r]["preds"]                       # [128, NT] u32
        e = res.results[r]["eta"]                         # [128, NT] f32
        preds[r * NS:(r + 1) * NS] = p.T.ravel().astype(np.int32)
        eta[r * NS:(r + 1) * NS] = e.T.ravel()
    return preds, eta


# revision 13
# speedup vs baseline: 1.1575x; 1.0241x over previous
"""HDModel retrieval kernel for 8x TRN2 NeuronCores.

reference:
    sims  = l2norm(hvs) @ l2norm(am).T        # [N, C] cosine sims
    preds = argmax(sims, axis=1)              # int32 [N]
    eta   = (sims[:,1]-sims[:,0])*0.25 + 0.5  # f32 [N]

Strategy (data-parallel over N, am replicated - no cross-core comms):
  - Host pre-transposes hvs -> hvsT [D, N/8] per shard and am -> amT [D, C]
    (layout staging only; all math happens on device).
  - sims computed as raw = hvsT.T @ amT with f32r (tf32) matmuls: the PE is
    the bottleneck engine at 16*32*1024 = 524288 col-streams (218.5us).
  - am column norms via ones-matmuls over ACT-squared am chunks; these are
    deliberately kept on the PE: they are am-gated filler that keeps the PE
    saturated during the serial 360GB/s am DMA window (the am stream is the
    binding resource for the first ~77us).
  - hvs row norms off the PE: ACT squares each hx half, DVE reduces over the
    dc axis (strided view), then a 4-cycle f32 matmul with a ones vector
    folds partitions; all 16 results land in one PSUM bank (bR).
  - PSUM: pA chains rotate over 4 single banks, pB chains over 3; pn /
    inv_c-broadcast / bR multiplex through the 8th bank. This lets tile 3
    start its pA chain the moment the last am chunk lands (no pair wait).
  - argmax top-8 indices from DVE max_index are written straight into a wide
    [128,128] accumulator; the host picks column 8*t (layout-only gather).
  - Emission order (= Tile scheduling priority) comes from a calibrated
    discrete-event estimate of each engine's in-order queue, so every queue
    chases the serial DMA stream without head-of-line stalls.

This walrus build encodes ONE sync wait per TPB instruction; Tile attaches
several, so a post-pass splits multi-wait instructions into single-wait
same-engine NoOps (see _split_multiwait).
"""
import numpy as np
from contextlib import ExitStack

import concourse.bass as bass
import concourse.mybir as mybir
import concourse.tile as tile
from concourse.bass_utils import run_bass_kernel_spmd

f32 = mybir.dt.float32
f32r = mybir.dt.float32r
bf16 = mybir.dt.bfloat16
u32 = mybir.dt.uint32

N_CORES = 8
N_FULL, D, C = 16384, 4096, 1024
NS = N_FULL // N_CORES          # 2048 rows per core
NT = NS // 128                  # 16 n-tiles
DCH = D // 128                  # 32 d-chunks
HD = DCH // 2                   # 16 d-chunks per hx half
QD = DCH // 4                   # 8 d-chunks per hx quarter
T1 = 3                          # n-tiles in flight during the am window

# calibrated cost-model estimates (ns), used only to choose emission order
E_AM = 1728.0        # am chunk DMA [128,1024] f32 (measured)
E_HX = 2913.0        # hx half DMA [128,2048] f32 (measured)
E_HXQ = 1456.0       # hx quarter DMA
E_MM = 427.0         # 2x512-col f32r matmul pair at 2.4GHz
E_SQAM = 1100.0      # ACT square of an am chunk
E_SQHX = 1960.0      # ACT square of an hx half
E_RED = 2300.0       # DVE dc-reduce of one squared hx half
SEM = 950.0          # DMA completion sem latency
DMA_T0 = 2300.0      # first-transfer start offset


def _split_multiwait(nc):
    """Split multi-wait instructions into single-wait NoOps (walrus limit)."""
    ctr = [0]

    def mk_nop(engine, wait=None, update=None):
        ctr[0] += 1
        nop = mybir.InstNoOp(name=f"mwsplit_{ctr[0]}", ins=[], outs=[])
        nop.engine = engine
        nop.sync_info = mybir.SyncInfo(
            on_wait=[wait] if wait is not None else [],
            on_update=[update] if update is not None else [],
        )
        return nop

    for f in nc.m.functions:
        for bb in f.blocks:
            new = []
            changed = False
            for inst in bb.instructions:
                si = inst.sync_info
                if si is None:
                    new.append(inst)
                    continue
                waits = list(si.on_wait)
                updates = list(si.on_update)
                pre, post = [], []
                if len(waits) > 1:
                    pre = [mk_nop(inst.engine, wait=w) for w in waits[:-1]]
                    waits = waits[-1:]
                if len(updates) > 1 and type(inst).__name__ != "InstDMACopy":
                    post = [mk_nop(inst.engine, update=u) for u in updates[1:]]
                    updates = updates[:1]
                if pre or post:
                    inst.sync_info = mybir.SyncInfo(on_wait=waits, on_update=updates)
                    new.extend(pre)
                    new.append(inst)
                    new.extend(post)
                    changed = True
                else:
                    new.append(inst)
            if changed:
                bb.instructions = new


def _phase1_dma_order():
    """Serial-DMA item order for the am window. hx tile 0 goes in quarters so
    the first matmul can start ~4us earlier; consts ride after the first few
    am chunks (their consumers have slack)."""
    order = ["q0", "a0", "q1", "a1", "q2", "a2", "q3", "a3",
             "cb", "cf", "cc", "zb", "zf", "zc", "a4", "a5",
             "h1A", "a6", "a7", "h1B", "a8", "a9", "a10",
             "h2A", "a11", "a12", "h2B"]
    order += [f"a{dc}" for dc in range(13, DCH)]
    return order


def build_nc():
    nc = bass.Bass()
    hvsT = nc.declare_dram_parameter("hvsT", [D, NS], f32r, isOutput=False)
    amT = nc.declare_dram_parameter("amT", [D, C], f32r, isOutput=False)
    ones_b = nc.declare_dram_parameter("ones_b", [128, 1], bf16, isOutput=False)
    ones_f = nc.declare_dram_parameter("ones_f", [128, 1], f32, isOutput=False)
    ones_c = nc.declare_dram_parameter("ones_c", [1, 128], f32, isOutput=False)
    zeros_b = nc.declare_dram_parameter("zeros_b", [128, 1], bf16, isOutput=False)
    zeros_f = nc.declare_dram_parameter("zeros_f", [128, 1], f32, isOutput=False)
    zeros_c = nc.declare_dram_parameter("zeros_c", [1, 128], f32, isOutput=False)
    preds_o = nc.declare_dram_parameter("preds", [128, 8 * NT], u32, isOutput=True)
    eta_o = nc.declare_dram_parameter("eta", [128, NT], f32, isOutput=True)

    with tile.TileContext(nc) as tc, ExitStack() as ctx:
        const_p = ctx.enter_context(tc.tile_pool(name="const", bufs=1))
        am_p = ctx.enter_context(tc.tile_pool(name="am", bufs=1))
        hx_p = ctx.enter_context(tc.tile_pool(name="hx", bufs=6))
        sqam_p = ctx.enter_context(tc.tile_pool(name="sqam", bufs=2))
        sqh_p = ctx.enter_context(tc.tile_pool(name="sqh", bufs=1))
        sqr_p = ctx.enter_context(tc.tile_pool(name="sqr", bufs=4))
        sc_p = ctx.enter_context(tc.tile_pool(name="sc", bufs=1))
        ep_p = ctx.enter_context(tc.tile_pool(name="ep", bufs=2))
        acc_p = ctx.enter_context(tc.tile_pool(name="acc", bufs=1))
        pa_p = ctx.enter_context(tc.tile_pool(name="pa", bufs=4, space="PSUM"))
        pb_p = ctx.enter_context(tc.tile_pool(name="pb", bufs=3, space="PSUM"))
        psn_p = ctx.enter_context(tc.tile_pool(name="psn", bufs=1, space="PSUM"))

        # ---------- event list: (est_start_ns, seq, emit_fn) ----------
        ev = []

        def emit(t_est, fn):
            ev.append((t_est, len(ev), fn))

        eng = {"dma": DMA_T0, "pe": 0.0, "act": 0.0, "dve": 0.0}

        def sched(engine, ready, dur, fn):
            t0 = max(eng[engine], ready)
            eng[engine] = t0 + dur
            emit(t0, fn)
            return t0 + dur

        # ---------- tiles ----------
        ones_t = const_p.tile([128, 1], bf16)
        ones_ft = const_p.tile([128, 1], f32)
        ones_ct = const_p.tile([1, 128], f32)
        zeros_t = const_p.tile([128, 1], bf16)
        zeros_ft = const_p.tile([128, 1], f32)
        zeros_ct = const_p.tile([1, 128], f32)
        am_tiles = [am_p.tile([128, C], f32r, tag=f"am{dc}", name=f"am{dc}")
                    for dc in range(DCH)]
        hx_half = {}                      # (t, h) -> tile
        inv_cb = const_p.tile([128, C], f32)
        inv_c = const_p.tile([1, C], f32)
        amn = inv_cb[0:1, :]              # scratch overlay, overwritten by bcast
        preds_acc = acc_p.tile([128, 8 * NT], u32)
        eta_acc = acc_p.tile([128, NT], f32)

        am_arr = {}                       # dc -> est data-ready (incl sem)
        hx_arr = {}                       # (t, h) -> est data-ready
        hxq_arr = {}                      # (t, dc) -> est ready, quarter-level
        sqam_done = {}
        red_done = {}
        tiny_done = {}
        mm_done = {}                      # t -> est of pB stop matmul
        free_a = {}                       # t -> est pA bank free (sc0 read)
        free_b = {}                       # t -> est pB bank free (sc1 read)
        d01_done = {}

        # ---------- DMA helpers ----------
        def dma_am(dc):
            def fn():
                nc.sync.dma_start(am_tiles[dc][:], amT[dc * 128:(dc + 1) * 128, :])
            am_arr[dc] = sched("dma", 0.0, E_AM, fn) + SEM

        def dma_hx(t, h):
            ht = hx_p.tile([128, HD * 128], f32r, tag="hx", name=f"hx{t}_{h}")
            hx_half[(t, h)] = ht

            def fn():
                src = hvsT[h * 2048:(h + 1) * 2048,
                           t * 128:(t + 1) * 128].rearrange(
                    "(dc p) j -> p dc j", p=128)
                nc.sync.dma_start(
                    ht[:].rearrange("p (dc j) -> p dc j", j=128), src)
            hx_arr[(t, h)] = sched("dma", 0.0, E_HX, fn) + SEM

        def dma_hx_quarter(t, h, q):
            """q in (0,1) within half h; tile allocated on q==0."""
            if q == 0:
                ht = hx_p.tile([128, HD * 128], f32r, tag="hx",
                               name=f"hx{t}_{h}")
                hx_half[(t, h)] = ht
            ht = hx_half[(t, h)]

            def fn():
                base = h * 2048 + q * 1024
                src = hvsT[base:base + 1024,
                           t * 128:(t + 1) * 128].rearrange(
                    "(dc p) j -> p dc j", p=128)
                dst = ht[:, q * QD * 128:(q + 1) * QD * 128]
                nc.sync.dma_start(dst.rearrange("p (dc j) -> p dc j", j=128), src)
            arr = sched("dma", 0.0, E_HXQ, fn) + SEM
            hx_arr[(t, h)] = arr          # half fully ready after last quarter
            return arr

        # ---------- ACT squares ----------
        sq_tiles = {}

        def act_sqam(dc):
            sq = sqam_p.tile([128, C], bf16, tag="sqam", name=f"sqam{dc}")
            sq_tiles[dc] = sq

            def fn():
                nc.scalar.square(sq[:], am_tiles[dc][:].bitcast(f32))
            sqam_done[dc] = sched("act", am_arr[dc], E_SQAM, fn)

        def rownorm_half(t, h):
            ht = hx_half[(t, h)]
            sq = sqh_p.tile([128, HD * 128], bf16, tag="sqh", name=f"sqh{t}_{h}")
            sqr = sqr_p.tile([128, 128], f32, tag="sqr", name=f"sqr{t}_{h}")

            def fn_sq():
                nc.scalar.square(sq[:], ht[:].bitcast(f32))
            t_sq = sched("act", hx_arr[(t, h)], E_SQHX, fn_sq)

            def fn_red():
                nc.vector.reduce_sum(
                    sqr[:], sq[:].rearrange("p (dc j) -> p j dc", j=128),
                    axis=mybir.AxisListType.X)
            t_red = sched("dve", t_sq, E_RED, fn_red)
            red_done[t] = max(red_done.get(t, 0.0), t_red)
            return sqr

        # bR is created at the boundary so the psn pool's single bank
        # multiplexes pn -> bcast scratch -> bR
        bR_ref = [None]

        def rownorm_mms(t, sqrA, sqrB):
            bR = bR_ref[0]

            def fnA():
                nc.tensor.matmul(bR[:, t:t + 1], sqrA[:], ones_ft[:],
                                 start=True, stop=False)

            def fnB():
                nc.tensor.matmul(bR[:, t:t + 1], sqrB[:], ones_ft[:],
                                 start=False, stop=False)

            def fnZ():
                nc.tensor.matmul(bR[:, t:t + 1], sqrB[:], zeros_ft[:],
                                 start=False, stop=True)
            t0 = sched("pe", max(red_done[t], mm_done.get(t, 0.0)), 10.0, fnA)
            t0 = sched("pe", t0, 10.0, fnB)
            tiny_done[t] = sched("pe", t0, 10.0, fnZ)

        # ---------- main matmuls ----------
        pas, pbs = {}, {}

        def get_banks(t):
            if t not in pas:
                pas[t] = pa_p.tile([128, 512], f32, tag="pa", name=f"pa{t}")
                pbs[t] = pb_p.tile([128, 512], f32, tag="pb", name=f"pb{t}")
            return pas[t], pbs[t]

        def mm(t, dc):
            pa, pb = get_banks(t)
            ht = hx_half[(t, dc // HD)]
            lhs = ht[:, (dc % HD) * 128:(dc % HD + 1) * 128]

            def fnA():
                nc.tensor.matmul(pa[:], lhs, am_tiles[dc][:, 0:512],
                                 start=(dc == 0), stop=(dc == DCH - 1))

            def fnB():
                nc.tensor.matmul(pb[:], lhs, am_tiles[dc][:, 512:C],
                                 start=(dc == 0), stop=(dc == DCH - 1))
            hx_ready = hxq_arr.get((t, dc), hx_arr[(t, dc // HD)])
            readyA = max(am_arr[dc], hx_ready, free_a.get(t - 4, 0.0))
            t0 = sched("pe", readyA, E_MM / 2, fnA)
            readyB = max(t0, free_b.get(t - 3, 0.0))
            mm_done[t] = sched("pe", readyB, E_MM / 2, fnB)

        # am-norm filler matmuls (pn accumulation over dc; partitions 0 / 32)
        pn = psn_p.tile([128, 512], f32, tag="pnb", name="pn")
        pn_end = [0.0]

        def pn_mm(dc):
            sq = sq_tiles[dc]

            def fnA():
                nc.tensor.matmul(pn[0:1, :], ones_t[:], sq[:, 0:512],
                                 start=(dc == 0), stop=False)

            def fnB():
                nc.tensor.matmul(pn[32:33, :], ones_t[:], sq[:, 512:C],
                                 start=(dc == 0), stop=False)
            t0 = sched("pe", sqam_done[dc], E_MM / 2, fnA)
            pn_end[0] = sched("pe", t0, E_MM / 2, fnB)
            if dc == DCH - 1:
                # zero-contribution stop pair: lets the data columns drain
                # through the PSUM write pipeline before the stop semaphore
                # releases the ACT sqrt reader
                def fnZA():
                    nc.tensor.matmul(pn[0:1, :], zeros_t[:], sq[:, 0:512],
                                     start=False, stop=True)

                def fnZB():
                    nc.tensor.matmul(pn[32:33, :], zeros_t[:], sq[:, 512:C],
                                     start=False, stop=True)
                t0 = sched("pe", pn_end[0], E_MM / 2, fnZA)
                pn_end[0] = sched("pe", t0, E_MM / 2, fnZB)

        # ---------- phase 1 ----------
        sqrs = {t: [None, None] for t in range(T1)}
        for item in _phase1_dma_order():
            if item[0] == "a":
                dc = int(item[1:])
                dma_am(dc)
                act_sqam(dc)
            elif item[0] == "q":
                q = int(item[1:])
                arr = dma_hx_quarter(0, q // 2, q % 2)
                for dc in range(q * QD, (q + 1) * QD):
                    hxq_arr[(0, dc)] = arr
                if q % 2 == 1:
                    sqrs[0][q // 2] = rownorm_half(0, q // 2)
            elif item[0] in ("c", "z"):
                src = {"cb": (ones_t, ones_b), "cf": (ones_ft, ones_f),
                       "cc": (ones_ct, ones_c), "zb": (zeros_t, zeros_b),
                       "zf": (zeros_ft, zeros_f), "zc": (zeros_ct, zeros_c)}[item]
                sched("dma", 0.0, 60.0,
                      lambda s=src: nc.sync.dma_start(s[0][:], s[1][:]))
            else:
                t, h = int(item[1:-1]), (0 if item[-1] == "A" else 1)
                dma_hx(t, h)
                sqrs[t][h] = rownorm_half(t, h)

        # chase-order the phase-1 PE stream
        cand = [("mm", t, dc) for t in range(T1) for dc in range(DCH)]
        cand += [("pn", 0, dc) for dc in range(DCH)]

        def ready_of(kind, t, dc):
            if kind == "pn":
                return sqam_done[dc]
            return max(am_arr[dc],
                       hxq_arr.get((t, dc), hx_arr.get((t, dc // HD), 0.0)))

        cand.sort(key=lambda c: (ready_of(*c), c[2], c[1]))
        for kind, t, dc in cand:
            if kind == "pn":
                pn_mm(dc)
            else:
                mm(t, dc)

        # ---------- boundary ----------
        def fn_sqrt_pn0():
            nc.scalar.sqrt(amn[:, 0:512], pn[0:1, :])

        def fn_sqrt_pn1():
            nc.scalar.sqrt(amn[:, 512:C], pn[32:33, :])

        def fn_recip_pn():
            nc.vector.reciprocal(inv_c[:], amn[:])
        t_invc = sched("act", pn_end[0] + 200.0, 500.0, fn_sqrt_pn0)
        t_invc = sched("act", t_invc, 500.0, fn_sqrt_pn1)
        t_invc = sched("dve", t_invc + 100.0, 1200.0, fn_recip_pn)

        # tile 3's hx + rownorm pipeline comes right after am31 in the stream
        dma_hx(3, 0)
        dma_hx(3, 1)
        sqr3 = (rownorm_half(3, 0), rownorm_half(3, 1))

        # t3 pA chain starts immediately on the fresh 4th pA bank, with the
        # two inv_cb broadcast matmuls woven in when their inputs are ready
        pa3, pb3 = get_banks(3)

        def mm3A(dc):
            ht = hx_half[(3, dc // HD)]
            lhs = ht[:, (dc % HD) * 128:(dc % HD + 1) * 128]

            def fnA():
                nc.tensor.matmul(pa3[:], lhs, am_tiles[dc][:, 0:512],
                                 start=(dc == 0), stop=(dc == DCH - 1))
            return sched("pe", hx_arr[(3, dc // HD)], E_MM / 2, fnA)

        def bcast(h, ready):
            def fn_bc():
                nc.tensor.matmul(pn[:], ones_ct[:],
                                 inv_c[:, h * 512:(h + 1) * 512],
                                 start=True, stop=False)

            def fn_bcz():
                nc.tensor.matmul(pn[:], zeros_ct[:],
                                 inv_c[:, h * 512:(h + 1) * 512],
                                 start=False, stop=True)

            def fn_cp():
                nc.scalar.copy(inv_cb[:, h * 512:(h + 1) * 512], pn[:])
            t0 = sched("pe", ready, 860.0, fn_bc)
            t0 = sched("pe", t0, 860.0, fn_bcz)
            return sched("act", t0 + 200.0, 600.0, fn_cp)

        t3a_end = 0.0
        bc_emitted = 0
        bc_ready = t_invc + 150.0
        inv_cb_done = 0.0
        for dc in range(DCH):
            t3a_end = mm3A(dc)
            if bc_emitted < 2 and eng["pe"] >= bc_ready:
                inv_cb_done = bcast(bc_emitted, bc_ready)
                bc_ready = inv_cb_done + 150.0
                bc_emitted += 1
        while bc_emitted < 2:
            inv_cb_done = bcast(bc_emitted, bc_ready)
            bc_ready = inv_cb_done + 150.0
            bc_emitted += 1

        # bR bank is free after the bcast copies; tiny row-norm mms for t0..2
        bR_ref[0] = psn_p.tile([128, 512], f32, tag="pnb", name="bR")
        for t in range(T1):
            rownorm_mms(t, sqrs[t][0], sqrs[t][1])

        # ---------- epilogue ----------
        def epilogue(t):
            pa, pb = pas.pop(t), pbs.pop(t)
            sc = sc_p.tile([128, C], f32, tag="sc", name=f"sc{t}")
            mx = ep_p.tile([128, 8], f32, tag="mx", name=f"mx{t}")
            d01 = ep_p.tile([128, 1], f32, tag="d01", name=f"d01{t}")
            inv4n = ep_p.tile([128, 1], f32, tag="inv4n", name=f"inv4n{t}")
            nrm4 = ep_p.tile([128, 1], f32, tag="nrm4", name=f"nrm4_{t}")
            ready = max(mm_done[t] + 200.0, inv_cb_done + SEM / 2)

            def fn_sc0():
                nc.vector.tensor_mul(sc[:, 0:512], pa[:], inv_cb[:, 0:512])

            def fn_sc1():
                nc.vector.tensor_mul(sc[:, 512:C], pb[:], inv_cb[:, 512:C])
            free_a[t] = sched("dve", ready, 800.0, fn_sc0)
            free_b[t] = sched("dve", free_a[t], 800.0, fn_sc1)

            def fn_max():
                nc.vector.max(out=mx[:], in_=sc[:])

            def fn_mix():
                nc.vector.max_index(out=preds_acc[:, 8 * t:8 * (t + 1)],
                                    in_max=mx[:], in_values=sc[:])

            def fn_d01():
                nc.vector.tensor_sub(d01[:], sc[:, 1:2], sc[:, 0:1])
            t0 = sched("dve", free_b[t], 1200.0, fn_max)
            t0 = sched("dve", t0, 1200.0, fn_mix)
            d01_done[t] = sched("dve", t0, 100.0, fn_d01)

            def fn_sq4():
                nc.scalar.activation(nrm4[:], bR_ref[0][:, t:t + 1],
                                     mybir.ActivationFunctionType.Sqrt,
                                     scale=16.0)

            def fn_rec4():
                nc.vector.reciprocal(inv4n[:], nrm4[:])

            def fn_eta():
                nc.scalar.activation(eta_acc[:, t:t + 1], d01[:],
                                     mybir.ActivationFunctionType.Copy,
                                     bias=0.5, scale=inv4n[:])
            t0 = sched("act", tiny_done[t] + 200.0, 300.0, fn_sq4)
            t0 = sched("dve", max(t0, d01_done[t]), 150.0, fn_rec4)
            sched("act", t0 + 150.0, 300.0, fn_eta)

        for t in range(T1):
            epilogue(t)

        # ---------- phase 2 ----------
        # t3: pB chain after its pA chain (pb bank frees via t0's sc1-mul)
        def mm3B(dc):
            ht = hx_half[(3, dc // HD)]
            lhs = ht[:, (dc % HD) * 128:(dc % HD + 1) * 128]

            def fnB():
                nc.tensor.matmul(pb3[:], lhs, am_tiles[dc][:, 512:C],
                                 start=(dc == 0), stop=(dc == DCH - 1))
            ready = max(t3a_end, free_b.get(0, 0.0)) if dc == 0 else 0.0
            return sched("pe", ready, E_MM / 2, fnB)

        for dc in range(DCH):
            mm_done[3] = mm3B(dc)
        rownorm_mms(3, *sqr3)
        epilogue(3)

        for t in range(4, NT):
            dma_hx(t, 0)
            dma_hx(t, 1)
            sqrA = rownorm_half(t, 0)
            sqrB = rownorm_half(t, 1)
            for dc in range(DCH):
                mm(t, dc)
            rownorm_mms(t, sqrA, sqrB)
            epilogue(t)

        # ---------- outputs ----------
        def fn_po():
            nc.sync.dma_start(preds_o[:], preds_acc[:])

        def fn_eo():
            nc.sync.dma_start(eta_o[:], eta_acc[:])
        t_end = max(eng["dve"], eng["act"])
        sched("dma", t_end, 100.0, fn_po)
        sched("dma", t_end, 100.0, fn_eo)

        # ---------- emit in estimated-start order ----------
        ev.sort(key=lambda e: (e[0], e[1]))
        for _, _, fn in ev:
            fn()

    _split_multiwait(nc)
    return nc


_CACHE = {}


def kernel(hvs: np.ndarray, am: np.ndarray):
    hvs = np.asarray(hvs, dtype=np.float32)
    am = np.asarray(am, dtype=np.float32)
    assert hvs.shape == (N_FULL, D) and am.shape == (C, D)

    if "nc" not in _CACHE:
        _CACHE["nc"] = build_nc()
    nc = _CACHE["nc"]

    amT = np.ascontiguousarray(am.T)                      # [D, C]
    import ml_dtypes
    ones_b = np.ones((128, 1), dtype=ml_dtypes.bfloat16)
    ones_f = np.ones((128, 1), dtype=np.float32)
    ones_c = np.ones((1, 128), dtype=np.float32)
    zeros_b = np.zeros((128, 1), dtype=ml_dtypes.bfloat16)
    zeros_f = np.zeros((128, 1), dtype=np.float32)
    zeros_c = np.zeros((1, 128), dtype=np.float32)

    in_maps = []
    for r in range(N_CORES):
        shard = hvs[r * NS:(r + 1) * NS]                  # [NS, D]
        hvsT = np.ascontiguousarray(shard.T)              # [D, NS]
        in_maps.append({"hvsT": hvsT, "amT": amT, "ones_b": ones_b,
                        "ones_f": ones_f, "ones_c": ones_c,
                        "zeros_b": zeros_b, "zeros_f": zeros_f,
                        "zeros_c": zeros_c})

    res = run_bass_kernel_spmd(nc, in_maps, core_ids=list(range(N_CORES)))

    preds = np.empty(N_FULL, dtype=np.int32)
    eta = np.empty(N_FULL, dtype=np.float32)
    for r in range(N_CORES):
        p = res.results[r]["preds"]                       # [128, 8*NT] u32
        e = res.results[r]["eta"]                         # [128, NT] f32
        preds[r * NS:(r + 1) * NS] = p[:, 0::8].T.ravel().astype(np.int32)
        eta[r * NS:(r + 1) * NS] = e.T.ravel()
    return preds, eta
